# revision 104
# baseline (speedup 1.0000x reference)
"""Trainium2 Bass kernel for nn_Attention_Embedding (spatial NxN attention +
channel CxC attention + conv3d(1,1,4) embedding head).

Sharding: 8 cores = 4 samples x 2 halves (split on H). Each core holds its
sample's full q (softmax rows are complete) and produces its own slice of the
final output; no cross-core communication.

Math per core (sample b, rows n in [n0, n0+2048)):
  big branch:   S_T[j,i] = sum_c q[j,c] q[n0+i,c]      (f32r matmuls)
                P_T = exp(S_T)                          (ACT, psum->sbuf, f32r)
                ca_aug_T[m,i] = sum_j q_aug[j,m] P_T[j,i]   (m=32 row = denom)
                caF_T = beta*ca_T/denom + q_loc_T
  small branch: G = q_aug^T q_aug (fp32) ; energy2 = wq_aug^T G wk_aug
                attn2 = softmax(energy2) ; V_T = wv_aug^T q_aug_loc^T
                paF_T = gamma * attn2 @ V_T + q_loc_T
  head:         conv3d(1,1,4) via im2col on partitions: X2T[32k+c, n] =
                X_T[c, n+k], one K=128 matmul per 512-slice; relu; branch sum;
                transpose back picking the 13 valid D-positions per block.
"""

import os
import sys

for _p in ("/opt/trn_rl_repo", "/root/.axon_site/_ro/trn_rl_repo"):
    if os.path.isdir(_p) and _p not in sys.path:
        sys.path.insert(0, _p)
        break

import ml_dtypes
import numpy as np

import concourse.bacc as bacc
import concourse.bass as bass
import concourse.mybir as mybir
import concourse.tile as tile
from concourse import bass_utils

B, H, W, D, C = 4, 16, 16, 16, 32
N = H * W * D            # 4096
NL = N // 2              # 2048 rows per core
DO = D - 3               # 13 conv output positions
NCORES = 8
NJC = N // 128           # 32 j-chunks
NIT = NL // 512          # 4 i-tiles of 512
HWL = NL // D            # 128 local (h,w) blocks
NOUT = HWL * DO          # 1664 output rows per core

f32 = mybir.dt.float32
f32r = mybir.dt.float32r
bf16 = mybir.dt.bfloat16
FT = mybir.ActivationFunctionType
ALU = mybir.AluOpType
PSUM = bass.MemorySpace.PSUM

# S_T matmul input dtype: f32r keeps softmax logits at ~1e-4; bf16 is ~2x
# faster on a cold PE clock but costs ~1e-3 in final accuracy.
S_BF16 = True
S_DT = bf16 if S_BF16 else f32r


def _conv_branch(tc, nc, psum_pool, wp, x4, w4x4, bias_v, relu_out, tag):
    """x4: (128, NL+4) f32r branch output replicated on 4 row-groups.
    conv3d(1,1,4) = 4 shift-accumulated flat matmuls per 512-column chunk,
    row-quad-packed (K=32): the 4 chunks of each k-shift run concurrently.
    Columns whose D-window crosses a block boundary land in the d' >= 13
    outputs, which the host discards."""
    cv_ps = psum_pool.tile([C, NL], f32, tag=tag)
    for k in range(4):
        for r in range(NIT):
            nc.tensor.matmul(
                cv_ps[:, r * 512:(r + 1) * 512],
                w4x4[32 * r:32 * r + C, k * C:(k + 1) * C],
                x4[32 * r:32 * r + C, r * 512 + k:r * 512 + k + 512],
                start=(k == 0), stop=(k == 3),
                tile_position=(32 * r, 0), skip_group_check=True,
            )
    for g in range(NIT):
        nc.scalar.activation(
            relu_out[:, g * 512:(g + 1) * 512], cv_ps[:, g * 512:(g + 1) * 512],
            FT.Relu, bias=bias_v[:],
        )


def _emit(tc, nc, t, out_d):
    with (
        tc.tile_pool(name="const", bufs=1) as cp,
        tc.tile_pool(name="work", bufs=1) as wp,
    ):
        # ---- load inputs to SBUF ----
        # q columns are host-permuted local-half-first, so S_T rhs slices and
        # lhsT j-chunks both come from qTP_r. Rows replicated x3 on-chip for
        # the row-trio-packed matmuls.
        qTP_r = cp.tile([96, N], S_DT)
        qTloc_r = cp.tile([C + 1, NL], f32r)
        qTloc_f = cp.tile([C, NL], f32)
        qc2_b = cp.tile([128, NJC, 128], bf16)  # [data|ones] replicated x2
        qc_f = cp.tile([128, NJC, C + 1], f32)
        wq_f = cp.tile([C + 1, C], f32)
        wk_f = cp.tile([C + 1, C], f32)
        wvT_r = cp.tile([C, C + 1], f32r)
        wch4 = cp.tile([128, 4 * C], f32r)
        wpos4 = cp.tile([128, 4 * C], f32r)
        bch_v = cp.tile([C, 1], f32)
        bpos_v = cp.tile([C, 1], f32)
        beta_v = cp.tile([1, 1], f32)
        beta128_v = cp.tile([128, 1], f32)
        gamma_v = cp.tile([C, 1], f32)
        ones_r = cp.tile([1, C], f32r)
        id32_r = cp.tile([C, C], f32r)
        id32_f = cp.tile([C, C], f32)
        # qcf first: the Gram matmuls are the PE's HAM warmup, start them ASAP.
        # Loads split across the sync and (idle) gpsimd queues.
        # sync's DMA queue is live ~7us before gpsimd's (engine boot), so the
        # Gram/trio-critical tensors go on sync; the rest ride gpsimd.
        nc.sync.dma_start(qc_f[:], t["qcf"])
        nc.sync.dma_start(qTP_r[0:C, :], t["qT"])
        nc.gpsimd.dma_start(qc2_b[:, :, 0:C], t["qc2d"])
        for name, tl in [
            ("qTloc", qTloc_r),
            ("wq", wq_f), ("wk", wk_f),
        ]:
            nc.sync.dma_start(tl[:], t[name])
        for name, tl in [
            ("wvT", wvT_r), ("wch4", wch4), ("wpos4", wpos4), ("bch", bch_v),
            ("bpos", bpos_v), ("beta", beta_v), ("beta128", beta128_v),
            ("gamma", gamma_v),
            ("ones", ones_r), ("id32r", id32_r), ("id32f", id32_f),
            ("qTlocf", qTloc_f),
        ]:
            nc.gpsimd.dma_start(tl[:], t[name])
        # trigger the ACT exp table load immediately (it costs ~2.7us and would
        # otherwise land on the attn2-softmax critical chain)
        warm = wp.tile([1, 1], f32)
        nc.scalar.activation(warm[:], wq_f[0:1, 0:1], FT.Exp)
        # replicate q^T onto row-groups 1 and 2 for the trio-packed matmuls
        nc.vector.tensor_copy(qTP_r[C:2 * C, :], qTP_r[0:C, :])
        nc.vector.tensor_copy(qTP_r[2 * C:3 * C, :], qTP_r[0:C, :])
        # qc2 = [data | ones] replicated onto both 64-column halves
        nc.vector.memset(qc2_b[:, :, C:2 * C], 1.0)
        nc.vector.tensor_copy(qc2_b[:, :, 2 * C:4 * C], qc2_b[:, :, 0:2 * C])

        relu_pos = wp.tile([C, NL], f32)

        # ============ Phase B (with the channel branch folded in) ============
        # S_T: matmuls row-trio-packed (K=32 -> 3 concurrent row-groups)
        # into (128, 1536) PSUM tiles; one big exp per tile (psum -> bf16 pt).
        # AV: bf16 col-pair-packed (M=64 = [data|ones]) accumulating 4 i-slices
        # into 2 stacked PSUM banks; ones columns give softmax denominators.
        # The small channel-attention branch is emitted as steps interleaved
        # between B tiles, borrowing "s"-tag PSUM slots; the Gram matmuls run
        # up front (PE warmup, overlapping the qTP DMA + replication) borrowing
        # the av0 bank before the AV accumulation first writes it.
        NSL = NJC * NIT          # 128 (jc, s) slice-matmuls
        NTL = (NSL + 2) // 3     # 43 psum tiles of up to 3 slices
        with tc.tile_pool(name="psAV", bufs=1, space=PSUM) as psAV:
            # -- whole channel branch (except its conv) runs pre-B on av0-tag
            # borrows, overlapping the qTP DMA + replication --
            g_ps = psAV.tile([C + 1, C + 1], f32, tag="av0")
            for jc in range(NJC):
                nc.tensor.matmul(
                    g_ps[:], qc_f[:, jc, :], qc_f[:, jc, :],
                    start=(jc == 0), stop=(jc == NJC - 1),
                )
            g_sb = wp.tile([C + 1, C + 1], f32)
            nc.vector.tensor_copy(g_sb[:], g_ps[:])
            # T1 = G @ wk_aug ; energy2 = wq_aug^T @ T1
            t1_ps = psAV.tile([C + 1, C], f32, tag="av0")
            nc.tensor.matmul(t1_ps[:], g_sb[:], wk_f[:], start=True, stop=True)
            t1_sb = wp.tile([C + 1, C], f32)
            nc.vector.tensor_copy(t1_sb[:], t1_ps[:])
            e2_ps = psAV.tile([C, C], f32, tag="av0")
            nc.tensor.matmul(e2_ps[:], wq_f[:], t1_sb[:], start=True, stop=True)
            # attn2 = softmax over free; energy2 spans ~[-290, 290]: max-sub
            mx = wp.tile([C, 1], f32)
            nc.vector.reduce_max(mx[:], e2_ps[:], axis=mybir.AxisListType.X)
            nmx = wp.tile([C, 1], f32)
            nc.vector.tensor_scalar_mul(nmx[:], mx[:], -1.0)
            a_sb = wp.tile([C, C], f32)
            nc.scalar.activation(a_sb[:], e2_ps[:], FT.Exp, bias=nmx[:])
            sm = wp.tile([C, 1], f32)
            nc.vector.reduce_sum(sm[:], a_sb[:], axis=mybir.AxisListType.X)
            rc = wp.tile([C, 1], f32)
            nc.vector.reciprocal(rc[:], sm[:])
            a_n = wp.tile([C, C], f32r)
            nc.vector.tensor_scalar_mul(a_n[:], a_sb[:], rc[:])
            at_ps = psAV.tile([C, C], f32, tag="av0")
            nc.tensor.matmul(at_ps[:], a_n[:], id32_r[:], start=True, stop=True)
            at_r = wp.tile([C, C], f32r)
            nc.vector.tensor_copy(at_r[:], at_ps[:])
            # wpa = wv_aug @ attn2^T, so pa_T = wpa @ q_loc_aug^T directly
            wpa_ps = psAV.tile([C + 1, C], f32, tag="av0")
            nc.tensor.matmul(wpa_ps[:], wvT_r[:], at_r[:], start=True, stop=True)
            wpa_r = wp.tile([C + 1, C], f32r)
            nc.vector.tensor_copy(wpa_r[:], wpa_ps[:])
            paF_r = wp.tile([128, NL + 4], f32r)
            nc.vector.tensor_scalar_mul(paF_r[0:C, NL:NL + 4], qTloc_f[:, 0:4], 0.0)
            for g in range(NIT):
                pa_ps = psAV.tile([C, 512], f32, tag="av0")
                nc.tensor.matmul(
                    pa_ps[:], wpa_r[:], qTloc_r[:, g * 512:(g + 1) * 512],
                    start=True, stop=True,
                )
                nc.vector.scalar_tensor_tensor(
                    paF_r[0:C, g * 512:(g + 1) * 512], pa_ps[:], gamma_v[:],
                    qTloc_f[:, g * 512:(g + 1) * 512],
                    op0=ALU.mult, op1=ALU.add,
                )
            # replicate onto row-groups 1..3 for the quad-packed conv
            for r in range(1, 4):
                eng = nc.sync if r == 2 else nc.gpsimd
                eng.dma_start(paF_r[32 * r:32 * r + C, :], paF_r[0:C, :])

            av0 = psAV.tile([128, 512], f32, tag="av0")  # slices 0,1
            av1 = psAV.tile([128, 512], f32, tag="av1")  # slices 2,3
            av_of = {0: (av0, 0), 1: (av0, 64), 2: (av1, 0), 3: (av1, 64)}

            def emit_av(jc, pt_slices):
                for s in range(NIT):
                    tile_, base = av_of[s]
                    pt_t, off = pt_slices[s]
                    nc.tensor.matmul(
                        tile_[base:base + 64, :],
                        qc2_b[:, jc, base:base + 64],
                        pt_t[:, off:off + 512],
                        start=(jc == 0), stop=(jc == NJC - 1),
                        tile_position=(0, base), skip_group_check=True,
                    )

            with (
                tc.tile_pool(name="psS", bufs=2, space=PSUM) as psS,
                tc.tile_pool(name="ptp", bufs=8) as ptp,
            ):
                slice_loc = {}
                done_jc = 0
                for tl_i in range(NTL):
                    idx0 = tl_i * 3
                    nsl = min(3, NSL - idx0)
                    s_ps = psS.tile([128, 1536], f32, tag="s")
                    for r in range(nsl):
                        jc, s = divmod(idx0 + r, NIT)
                        nc.tensor.matmul(
                            s_ps[:, r * 512:(r + 1) * 512],
                            qTP_r[32 * r:32 * r + C, jc * 128:(jc + 1) * 128],
                            qTP_r[32 * r:32 * r + C, s * 512:(s + 1) * 512],
                            start=True, stop=True,
                            tile_position=(32 * r, 0), skip_group_check=True,
                        )
                    pt = ptp.tile([128, 1536], bf16, tag="pt")
                    nc.scalar.activation(
                        pt[:, 0:nsl * 512], s_ps[:, 0:nsl * 512], FT.Exp
                    )
                    for r in range(nsl):
                        slice_loc[idx0 + r] = (pt, r * 512)
                    while done_jc < NJC and (done_jc + 1) * NIT <= idx0 + nsl:
                        emit_av(done_jc,
                                [slice_loc[done_jc * NIT + s] for s in range(NIT)])
                        done_jc += 1

            # ---- conv_pos (fills the PE while the normalization chain runs)
            # and normalize: caF_T = beta * ca_T / denom + q_loc_T ----
            # av layout: slice s -> (av_{s//2}, base 64*(s%2)): rows base..base+32
            # hold ca columns, rows base+32..base+64 hold the denominators.
            with (
                tc.tile_pool(name="psC1", bufs=1, space=PSUM) as psC1,
                tc.tile_pool(name="nrm", bufs=3) as nrm,
            ):
                _conv_branch(tc, nc, psC1, wp, paF_r, wpos4, bpos_v, relu_pos, "cvp")
                # denominators: psum -> DRAM -> broadcast-read as a (128, 512)
                # stack (32 partitions per slice) so reciprocal runs 512/lane
                with tc.tile_pool(name="drp", bufs=1, space=bass.MemorySpace.DRAM) as drp:
                    sums_d = drp.tile([NIT, 512], f32)
                    for s in range(NIT):
                        tile_, base = av_of[s]
                        ssb = nrm.tile([1, 512], f32, tag="ssb")
                        nc.scalar.copy(ssb[:], tile_[base + C:base + C + 1, :])
                        eng = nc.sync if s % 2 == 0 else nc.gpsimd
                        eng.dma_start(sums_d[s:s + 1, :], ssb[:])
                    rec_in = wp.tile([128, 512], f32)
                    for s in range(NIT):
                        eng = nc.sync if s % 2 == 0 else nc.gpsimd
                        eng.dma_start(
                            rec_in[32 * s:32 * (s + 1), :],
                            sums_d[s:s + 1, :].partition_broadcast(C),
                        )
                rec = wp.tile([128, 512], f32)
                nc.vector.reciprocal_approx_fast(rec[:], rec_in[:])
                recB = wp.tile([128, 512], f32)
                nc.vector.tensor_scalar_mul(recB[:], rec[:], beta128_v[:])
                caF_r = wp.tile([128, NL + 4], f32r)
                nc.vector.tensor_scalar_mul(caF_r[0:C, NL:NL + 4], qTloc_f[:, 0:4], 0.0)
                tmp_full = wp.tile([C, NL], f32)
                for s in range(NIT):
                    tile_, base = av_of[s]
                    nc.vector.tensor_mul(
                        tmp_full[:, s * 512:(s + 1) * 512],
                        tile_[base:base + C, :],
                        recB[32 * s:32 * (s + 1), :],
                    )
                nc.vector.tensor_add(caF_r[0:C, 0:NL], tmp_full[:], qTloc_f[:])
                for r in (1, 2):
                    nc.vector.tensor_copy(
                        caF_r[C * r:C * 2 * r, :], caF_r[0:C * r, :]
                    )

        # ================= Phase C: conv head + output =================
        with (
            tc.tile_pool(name="psC2", bufs=1, space=PSUM) as psC2,
            tc.tile_pool(name="obp", bufs=4) as obp,
        ):
            relu_ch = wp.tile([C, NL], f32)
            _conv_branch(tc, nc, psC2, wp, caF_r, wch4, bch_v, relu_ch, "cvc")
            sumT = wp.tile([C, NL], f32)
            # Transpose back via DVE 32x32 block-transpose + strided DMA:
            # tb = [B0^T|B1^T|B2^T|B3^T] per 128-column chunk; the DMA scatters
            # block k to rows 128*tch + 32k. d' in {13,14,15} rows are garbage,
            # host drops them.
            out_v = out_d.rearrange("(g kk r) f -> g r kk f", kk=16, r=C)
            for g in range(NIT):
                nc.vector.tensor_add(
                    sumT[:, g * 512:(g + 1) * 512],
                    relu_ch[:, g * 512:(g + 1) * 512],
                    relu_pos[:, g * 512:(g + 1) * 512],
                )
                tb = obp.tile([C, 512], f32, tag="ob")
                nc.vector.transpose(tb[:], sumT[:, g * 512:(g + 1) * 512])
                eng = nc.sync if g % 2 == 0 else nc.gpsimd
                eng.dma_start(
                    out_v[g],
                    tb[:].rearrange("r (kk f) -> r kk f", kk=16),
                )


def _build():
    nc = bacc.Bacc("TRN2", target_bir_lowering=False, debug=False)
    t = {}

    def din(name, shape, dt):
        t[name] = nc.dram_tensor(name, shape, dt, kind="ExternalInput").ap()

    din("qT", [C, N], S_DT)
    din("qTloc", [C + 1, NL], f32r)
    din("qTlocf", [C, NL], f32)
    din("qc2d", [128, NJC, C], bf16)
    din("qcf", [128, NJC, C + 1], f32)
    din("wq", [C + 1, C], f32)
    din("wk", [C + 1, C], f32)
    din("wvT", [C, C + 1], f32r)
    din("wch4", [128, 4 * C], f32r)
    din("wpos4", [128, 4 * C], f32r)
    din("bch", [C, 1], f32)
    din("bpos", [C, 1], f32)
    din("beta", [1, 1], f32)
    din("beta128", [128, 1], f32)
    din("gamma", [C, 1], f32)
    din("ones", [1, C], f32r)
    din("id32r", [C, C], f32r)
    din("id32f", [C, C], f32)
    out_d = nc.dram_tensor("out", [NL, C], f32, kind="ExternalOutput").ap()

    with tile.TileContext(nc) as tc:
        _emit(tc, nc, t, out_d)
    nc.compile()
    return nc


_NC = None


def _get_nc():
    global _NC
    if _NC is None:
        _NC = _build()
    return _NC


def _prepare_in_maps(inputs):
    x = np.asarray(inputs["inputs"], np.float32)
    beta = np.asarray(inputs["beta"], np.float32)
    gamma = np.asarray(inputs["gamma"], np.float32)
    wq_aug = np.concatenate(
        [np.asarray(inputs["wq"], np.float32), np.asarray(inputs["bq"], np.float32)[None, :]], 0
    )
    wk_aug = np.concatenate(
        [np.asarray(inputs["wk"], np.float32), np.asarray(inputs["bk"], np.float32)[None, :]], 0
    )
    wv_aug = np.concatenate(
        [np.asarray(inputs["wv"], np.float32), np.asarray(inputs["bv"], np.float32)[None, :]], 0
    )
    shared = {
        "wq": wq_aug, "wk": wk_aug, "wvT": np.ascontiguousarray(wv_aug.T),
        "wch4": np.ascontiguousarray(np.tile(
            np.asarray(inputs["w_ch"], np.float32).reshape(4, C, C).transpose(1, 0, 2).reshape(C, 4 * C),
            (4, 1),
        )),
        "wpos4": np.ascontiguousarray(np.tile(
            np.asarray(inputs["w_pos"], np.float32).reshape(4, C, C).transpose(1, 0, 2).reshape(C, 4 * C),
            (4, 1),
        )),
        "bch": np.ascontiguousarray(np.asarray(inputs["b_ch"], np.float32)[:, None]),
        "bpos": np.ascontiguousarray(np.asarray(inputs["b_pos"], np.float32)[:, None]),
        "beta": np.full((1, 1), beta[0], np.float32),
        "beta128": np.full((128, 1), beta[0], np.float32),
        "gamma": np.full((C, 1), gamma[0], np.float32),
        "ones": np.ones((1, C), np.float32),
        "id32r": np.eye(C, dtype=np.float32),
        "id32f": np.eye(C, dtype=np.float32),
    }
    in_maps = []
    for core in range(NCORES):
        b, s = core // 2, core % 2
        qs = x[b].reshape(N, C)
        # local-half-first column permutation: S_T rhs slices [0, NL) are the
        # core's own rows; softmax sums over all j are order-invariant.
        q = np.concatenate([qs[s * NL:(s + 1) * NL], qs[(1 - s) * NL:(2 - s) * NL]])
        q_aug = np.concatenate([q, np.ones((N, 1), np.float32)], 1)
        qloc_aug = q_aug[:NL]
        qc = np.ascontiguousarray(q_aug.reshape(NJC, 128, C + 1).transpose(1, 0, 2))
        m = dict(shared)
        qT_host = np.ascontiguousarray(q.T)
        m["qT"] = qT_host.astype(ml_dtypes.bfloat16) if S_BF16 else qT_host
        m["qTloc"] = np.ascontiguousarray(qloc_aug.T)
        m["qTlocf"] = np.ascontiguousarray(qloc_aug.T[:C])
        m["qc2d"] = np.ascontiguousarray(qc[:, :, :C]).astype(ml_dtypes.bfloat16)
        m["qcf"] = qc
        in_maps.append(m)
    return in_maps


def _run(inputs, trace=False):
    nc = _get_nc()
    in_maps = _prepare_in_maps(inputs)
    res = bass_utils.run_bass_kernel_spmd(
        nc, in_maps, core_ids=list(range(NCORES)), trace=trace
    )
    out = np.empty((B, H, W, DO, C), np.float32)
    for core in range(NCORES):
        b, s = core // 2, core % 2
        full = res.results[core]["out"].reshape(8, W, D, C)
        out[b, s * 8:(s + 1) * 8] = full[:, :, :DO, :]
    return out, res


def kernel(**inputs):
    out, _ = _run(inputs, trace=False)
    return out


# revision 105
# speedup vs baseline: 1.0505x; 1.0505x over previous
"""Trainium2 Bass kernel for nn_Attention_Embedding (spatial NxN attention +
channel CxC attention + conv3d(1,1,4) embedding head).

Sharding: 8 cores = 4 samples x 2 halves (split on H). Each core holds its
sample's full q (softmax rows are complete) and produces its own slice of the
final output; no cross-core communication.

Math per core (sample b, rows n in [n0, n0+2048)):
  big branch:   S_T[j,i] = sum_c q[j,c] q[n0+i,c]      (f32r matmuls)
                P_T = exp(S_T)                          (ACT, psum->sbuf, f32r)
                ca_aug_T[m,i] = sum_j q_aug[j,m] P_T[j,i]   (m=32 row = denom)
                caF_T = beta*ca_T/denom + q_loc_T
  small branch: G = q_aug^T q_aug (fp32) ; energy2 = wq_aug^T G wk_aug
                attn2 = softmax(energy2) ; V_T = wv_aug^T q_aug_loc^T
                paF_T = gamma * attn2 @ V_T + q_loc_T
  head:         conv3d(1,1,4) via im2col on partitions: X2T[32k+c, n] =
                X_T[c, n+k], one K=128 matmul per 512-slice; relu; branch sum;
                transpose back picking the 13 valid D-positions per block.
"""

import os
import sys

for _p in ("/opt/trn_rl_repo", "/root/.axon_site/_ro/trn_rl_repo"):
    if os.path.isdir(_p) and _p not in sys.path:
        sys.path.insert(0, _p)
        break

import ml_dtypes
import numpy as np

import concourse.bacc as bacc
import concourse.bass as bass
import concourse.mybir as mybir
import concourse.tile as tile
from concourse import bass_utils

B, H, W, D, C = 4, 16, 16, 16, 32
N = H * W * D            # 4096
NL = N // 2              # 2048 rows per core
DO = D - 3               # 13 conv output positions
NCORES = 8
NJC = N // 128           # 32 j-chunks
NIT = NL // 512          # 4 i-tiles of 512
HWL = NL // D            # 128 local (h,w) blocks
NOUT = HWL * DO          # 1664 output rows per core

f32 = mybir.dt.float32
f32r = mybir.dt.float32r
bf16 = mybir.dt.bfloat16
FT = mybir.ActivationFunctionType
ALU = mybir.AluOpType
PSUM = bass.MemorySpace.PSUM

# S_T matmul input dtype: f32r keeps softmax logits at ~1e-4; bf16 is ~2x
# faster on a cold PE clock but costs ~1e-3 in final accuracy.
S_BF16 = True
S_DT = bf16 if S_BF16 else f32r


def _conv_branch(tc, nc, psum_pool, wp, x4, w4x4, bias_v, relu_out, tag):
    """x4: (128, NL+4) f32r branch output replicated on 4 row-groups.
    conv3d(1,1,4) = 4 shift-accumulated flat matmuls per 512-column chunk,
    row-quad-packed (K=32): the 4 chunks of each k-shift run concurrently.
    Columns whose D-window crosses a block boundary land in the d' >= 13
    outputs, which the host discards."""
    cv_ps = psum_pool.tile([C, NL], f32, tag=tag)
    for k in range(4):
        for r in range(NIT):
            nc.tensor.matmul(
                cv_ps[:, r * 512:(r + 1) * 512],
                w4x4[32 * r:32 * r + C, k * C:(k + 1) * C],
                x4[32 * r:32 * r + C, r * 512 + k:r * 512 + k + 512],
                start=(k == 0), stop=(k == 3),
                tile_position=(32 * r, 0), skip_group_check=True,
            )
    for g in range(NIT):
        nc.scalar.activation(
            relu_out[:, g * 512:(g + 1) * 512], cv_ps[:, g * 512:(g + 1) * 512],
            FT.Relu, bias=bias_v[:],
        )


def _emit(tc, nc, t, out_d):
    with (
        tc.tile_pool(name="const", bufs=1) as cp,
        tc.tile_pool(name="work", bufs=1) as wp,
    ):
        # ---- load inputs to SBUF ----
        # q columns are host-permuted local-half-first, so S_T rhs slices and
        # lhsT j-chunks both come from qTP_r. Rows replicated x3 on-chip for
        # the row-trio-packed matmuls.
        qTP_r = cp.tile([96, N], S_DT)
        qTloc_r = cp.tile([C + 1, NL], f32r)
        qTloc_f = cp.tile([C, NL], f32)
        qc2_b = cp.tile([128, NJC, 128], bf16)  # [data|ones] replicated x2
        qc_f = cp.tile([128, NJC, C + 1], f32)
        wq_f = cp.tile([C + 1, C], f32)
        wk_f = cp.tile([C + 1, C], f32)
        wvT_r = cp.tile([C, C + 1], f32r)
        wch4 = cp.tile([128, 4 * C], f32r)
        wpos4 = cp.tile([128, 4 * C], f32r)
        bch_v = cp.tile([C, 1], f32)
        bpos_v = cp.tile([C, 1], f32)
        beta_v = cp.tile([1, 1], f32)
        beta128_v = cp.tile([128, 1], f32)
        gamma_v = cp.tile([C, 1], f32)
        ones_r = cp.tile([1, C], f32r)
        id32_r = cp.tile([C, C], f32r)
        id32_f = cp.tile([C, C], f32)
        # qcf first: the Gram matmuls are the PE's HAM warmup, start them ASAP.
        # Loads split across the sync and (idle) gpsimd queues.
        nc.gpsimd.dma_start(qc_f[:], t["qcf"])
        nc.sync.dma_start(qTP_r[0:C, :], t["qT"])
        nc.gpsimd.dma_start(qc2_b[:, :, 0:C], t["qc2d"])
        for name, tl in [
            ("qTloc", qTloc_r),
            ("wq", wq_f), ("wk", wk_f),
            ("wvT", wvT_r), ("wch4", wch4), ("wpos4", wpos4), ("bch", bch_v),
            ("bpos", bpos_v), ("beta", beta_v), ("beta128", beta128_v),
            ("gamma", gamma_v),
            ("ones", ones_r), ("id32r", id32_r), ("id32f", id32_f),
        ]:
            nc.sync.dma_start(tl[:], t[name])
        nc.gpsimd.dma_start(qTloc_f[:], t["qTlocf"])
        # trigger the ACT exp table load immediately (it costs ~2.7us and would
        # otherwise land on the attn2-softmax critical chain)
        warm = wp.tile([1, 1], f32)
        nc.scalar.activation(warm[:], beta_v[:], FT.Exp)
        # replicate q^T onto row-groups 1 and 2 for the trio-packed matmuls
        nc.vector.tensor_copy(qTP_r[C:2 * C, :], qTP_r[0:C, :])
        nc.vector.tensor_copy(qTP_r[2 * C:3 * C, :], qTP_r[0:C, :])
        # qc2 = [data | ones] replicated onto both 64-column halves
        nc.vector.memset(qc2_b[:, :, C:2 * C], 1.0)
        nc.vector.tensor_copy(qc2_b[:, :, 2 * C:4 * C], qc2_b[:, :, 0:2 * C])

        relu_pos = wp.tile([C, NL], f32)

        # ============ Phase B (with the channel branch folded in) ============
        # S_T: matmuls row-trio-packed (K=32 -> 3 concurrent row-groups)
        # into (128, 1536) PSUM tiles; one big exp per tile (psum -> bf16 pt).
        # AV: bf16 col-pair-packed (M=64 = [data|ones]) accumulating 4 i-slices
        # into 2 stacked PSUM banks; ones columns give softmax denominators.
        # The small channel-attention branch is emitted as steps interleaved
        # between B tiles, borrowing "s"-tag PSUM slots; the Gram matmuls run
        # up front (PE warmup, overlapping the qTP DMA + replication) borrowing
        # the av0 bank before the AV accumulation first writes it.
        NSL = NJC * NIT          # 128 (jc, s) slice-matmuls
        NTL = (NSL + 2) // 3     # 43 psum tiles of up to 3 slices
        with tc.tile_pool(name="psAV", bufs=1, space=PSUM) as psAV:
            # -- whole channel branch (except its conv) runs pre-B on av0-tag
            # borrows, overlapping the qTP DMA + replication --
            g_ps = psAV.tile([C + 1, C + 1], f32, tag="av0")
            for jc in range(NJC):
                nc.tensor.matmul(
                    g_ps[:], qc_f[:, jc, :], qc_f[:, jc, :],
                    start=(jc == 0), stop=(jc == NJC - 1),
                )
            g_sb = wp.tile([C + 1, C + 1], f32)
            nc.vector.tensor_copy(g_sb[:], g_ps[:])
            # T1 = G @ wk_aug ; energy2 = wq_aug^T @ T1
            t1_ps = psAV.tile([C + 1, C], f32, tag="av0")
            nc.tensor.matmul(t1_ps[:], g_sb[:], wk_f[:], start=True, stop=True)
            t1_sb = wp.tile([C + 1, C], f32)
            nc.vector.tensor_copy(t1_sb[:], t1_ps[:])
            e2_ps = psAV.tile([C, C], f32, tag="av0")
            nc.tensor.matmul(e2_ps[:], wq_f[:], t1_sb[:], start=True, stop=True)
            # attn2 = softmax over free; energy2 spans ~[-290, 290]: max-sub
            mx = wp.tile([C, 1], f32)
            nc.vector.reduce_max(mx[:], e2_ps[:], axis=mybir.AxisListType.X)
            nmx = wp.tile([C, 1], f32)
            nc.vector.tensor_scalar_mul(nmx[:], mx[:], -1.0)
            a_sb = wp.tile([C, C], f32)
            nc.scalar.activation(a_sb[:], e2_ps[:], FT.Exp, bias=nmx[:])
            sm = wp.tile([C, 1], f32)
            nc.vector.reduce_sum(sm[:], a_sb[:], axis=mybir.AxisListType.X)
            rc = wp.tile([C, 1], f32)
            nc.vector.reciprocal(rc[:], sm[:])
            a_n = wp.tile([C, C], f32r)
            nc.vector.tensor_scalar_mul(a_n[:], a_sb[:], rc[:])
            at_ps = psAV.tile([C, C], f32, tag="av0")
            nc.tensor.matmul(at_ps[:], a_n[:], id32_r[:], start=True, stop=True)
            at_r = wp.tile([C, C], f32r)
            nc.vector.tensor_copy(at_r[:], at_ps[:])
            # wpa = wv_aug @ attn2^T, so pa_T = wpa @ q_loc_aug^T directly
            wpa_ps = psAV.tile([C + 1, C], f32, tag="av0")
            nc.tensor.matmul(wpa_ps[:], wvT_r[:], at_r[:], start=True, stop=True)
            wpa_r = wp.tile([C + 1, C], f32r)
            nc.vector.tensor_copy(wpa_r[:], wpa_ps[:])
            paF_r = wp.tile([128, NL + 4], f32r)
            nc.vector.tensor_scalar_mul(paF_r[0:C, NL:NL + 4], qTloc_f[:, 0:4], 0.0)
            for g in range(NIT):
                pa_ps = psAV.tile([C, 512], f32, tag="av0")
                nc.tensor.matmul(
                    pa_ps[:], wpa_r[:], qTloc_r[:, g * 512:(g + 1) * 512],
                    start=True, stop=True,
                )
                nc.vector.scalar_tensor_tensor(
                    paF_r[0:C, g * 512:(g + 1) * 512], pa_ps[:], gamma_v[:],
                    qTloc_f[:, g * 512:(g + 1) * 512],
                    op0=ALU.mult, op1=ALU.add,
                )
            # replicate onto row-groups 1..3 for the quad-packed conv
            for r in range(1, 4):
                eng = nc.sync if r == 2 else nc.gpsimd
                eng.dma_start(paF_r[32 * r:32 * r + C, :], paF_r[0:C, :])

            av0 = psAV.tile([128, 512], f32, tag="av0")  # slices 0,1
            av1 = psAV.tile([128, 512], f32, tag="av1")  # slices 2,3
            av_of = {0: (av0, 0), 1: (av0, 64), 2: (av1, 0), 3: (av1, 64)}

            def emit_av(jc, pt_slices):
                for s in range(NIT):
                    tile_, base = av_of[s]
                    pt_t, off = pt_slices[s]
                    nc.tensor.matmul(
                        tile_[base:base + 64, :],
                        qc2_b[:, jc, base:base + 64],
                        pt_t[:, off:off + 512],
                        start=(jc == 0), stop=(jc == NJC - 1),
                        tile_position=(0, base), skip_group_check=True,
                    )

            with (
                tc.tile_pool(name="psS", bufs=2, space=PSUM) as psS,
                tc.tile_pool(name="ptp", bufs=8) as ptp,
            ):
                slice_loc = {}
                done_jc = 0
                for tl_i in range(NTL):
                    idx0 = tl_i * 3
                    nsl = min(3, NSL - idx0)
                    s_ps = psS.tile([128, 1536], f32, tag="s")
                    for r in range(nsl):
                        jc, s = divmod(idx0 + r, NIT)
                        nc.tensor.matmul(
                            s_ps[:, r * 512:(r + 1) * 512],
                            qTP_r[32 * r:32 * r + C, jc * 128:(jc + 1) * 128],
                            qTP_r[32 * r:32 * r + C, s * 512:(s + 1) * 512],
                            start=True, stop=True,
                            tile_position=(32 * r, 0), skip_group_check=True,
                        )
                    pt = ptp.tile([128, 1536], bf16, tag="pt")
                    nc.scalar.activation(
                        pt[:, 0:nsl * 512], s_ps[:, 0:nsl * 512], FT.Exp
                    )
                    for r in range(nsl):
                        slice_loc[idx0 + r] = (pt, r * 512)
                    while done_jc < NJC and (done_jc + 1) * NIT <= idx0 + nsl:
                        emit_av(done_jc,
                                [slice_loc[done_jc * NIT + s] for s in range(NIT)])
                        done_jc += 1

            # ---- conv_pos (fills the PE while the normalization chain runs)
            # and normalize: caF_T = beta * ca_T / denom + q_loc_T ----
            # av layout: slice s -> (av_{s//2}, base 64*(s%2)): rows base..base+32
            # hold ca columns, rows base+32..base+64 hold the denominators.
            with (
                tc.tile_pool(name="psC1", bufs=1, space=PSUM) as psC1,
                tc.tile_pool(name="nrm", bufs=3) as nrm,
            ):
                _conv_branch(tc, nc, psC1, wp, paF_r, wpos4, bpos_v, relu_pos, "cvp")
                # denominators: psum -> DRAM -> broadcast-read as a (128, 512)
                # stack (32 partitions per slice) so reciprocal runs 512/lane
                with tc.tile_pool(name="drp", bufs=1, space=bass.MemorySpace.DRAM) as drp:
                    sums_d = drp.tile([NIT, 512], f32)
                    for s in range(NIT):
                        tile_, base = av_of[s]
                        ssb = nrm.tile([1, 512], f32, tag="ssb")
                        nc.scalar.copy(ssb[:], tile_[base + C:base + C + 1, :])
                        eng = nc.sync if s % 2 == 0 else nc.gpsimd
                        eng.dma_start(sums_d[s:s + 1, :], ssb[:])
                    rec_in = wp.tile([128, 512], f32)
                    for s in range(NIT):
                        eng = nc.sync if s % 2 == 0 else nc.gpsimd
                        eng.dma_start(
                            rec_in[32 * s:32 * (s + 1), :],
                            sums_d[s:s + 1, :].partition_broadcast(C),
                        )
                rec = wp.tile([128, 512], f32)
                nc.vector.reciprocal_approx_fast(rec[:], rec_in[:])
                recB = wp.tile([128, 512], f32)
                nc.vector.tensor_scalar_mul(recB[:], rec[:], beta128_v[:])
                caF_r = wp.tile([128, NL + 4], f32r)
                nc.vector.tensor_scalar_mul(caF_r[0:C, NL:NL + 4], qTloc_f[:, 0:4], 0.0)
                tmp_full = wp.tile([C, NL], f32)
                for s in range(NIT):
                    tile_, base = av_of[s]
                    nc.vector.tensor_mul(
                        tmp_full[:, s * 512:(s + 1) * 512],
                        tile_[base:base + C, :],
                        recB[32 * s:32 * (s + 1), :],
                    )
                nc.vector.tensor_add(caF_r[0:C, 0:NL], tmp_full[:], qTloc_f[:])
                for r in (1, 2):
                    nc.vector.tensor_copy(
                        caF_r[C * r:C * 2 * r, :], caF_r[0:C * r, :]
                    )

        # ================= Phase C: conv head + output =================
        with (
            tc.tile_pool(name="psC2", bufs=1, space=PSUM) as psC2,
            tc.tile_pool(name="obp", bufs=4) as obp,
        ):
            relu_ch = wp.tile([C, NL], f32)
            _conv_branch(tc, nc, psC2, wp, caF_r, wch4, bch_v, relu_ch, "cvc")
            sumT = wp.tile([C, NL], f32)
            # Transpose back via DVE 32x32 block-transpose + strided DMA:
            # tb = [B0^T|B1^T|B2^T|B3^T] per 128-column chunk; the DMA scatters
            # block k to rows 128*tch + 32k. d' in {13,14,15} rows are garbage,
            # host drops them.
            out_v = out_d.rearrange("(g kk r) f -> g r kk f", kk=16, r=C)
            for g in range(NIT):
                nc.vector.tensor_add(
                    sumT[:, g * 512:(g + 1) * 512],
                    relu_ch[:, g * 512:(g + 1) * 512],
                    relu_pos[:, g * 512:(g + 1) * 512],
                )
                tb = obp.tile([C, 512], f32, tag="ob")
                nc.vector.transpose(tb[:], sumT[:, g * 512:(g + 1) * 512])
                eng = nc.sync if g % 2 == 0 else nc.gpsimd
                eng.dma_start(
                    out_v[g],
                    tb[:].rearrange("r (kk f) -> r kk f", kk=16),
                )


def _build():
    nc = bacc.Bacc("TRN2", target_bir_lowering=False, debug=False)
    t = {}

    def din(name, shape, dt):
        t[name] = nc.dram_tensor(name, shape, dt, kind="ExternalInput").ap()

    din("qT", [C, N], S_DT)
    din("qTloc", [C + 1, NL], f32r)
    din("qTlocf", [C, NL], f32)
    din("qc2d", [128, NJC, C], bf16)
    din("qcf", [128, NJC, C + 1], f32)
    din("wq", [C + 1, C], f32)
    din("wk", [C + 1, C], f32)
    din("wvT", [C, C + 1], f32r)
    din("wch4", [128, 4 * C], f32r)
    din("wpos4", [128, 4 * C], f32r)
    din("bch", [C, 1], f32)
    din("bpos", [C, 1], f32)
    din("beta", [1, 1], f32)
    din("beta128", [128, 1], f32)
    din("gamma", [C, 1], f32)
    din("ones", [1, C], f32r)
    din("id32r", [C, C], f32r)
    din("id32f", [C, C], f32)
    out_d = nc.dram_tensor("out", [NL, C], f32, kind="ExternalOutput").ap()

    with tile.TileContext(nc) as tc:
        _emit(tc, nc, t, out_d)
    nc.compile()
    return nc


_NC = None


def _get_nc():
    global _NC
    if _NC is None:
        _NC = _build()
    return _NC


def _prepare_in_maps(inputs):
    x = np.asarray(inputs["inputs"], np.float32)
    beta = np.asarray(inputs["beta"], np.float32)
    gamma = np.asarray(inputs["gamma"], np.float32)
    wq_aug = np.concatenate(
        [np.asarray(inputs["wq"], np.float32), np.asarray(inputs["bq"], np.float32)[None, :]], 0
    )
    wk_aug = np.concatenate(
        [np.asarray(inputs["wk"], np.float32), np.asarray(inputs["bk"], np.float32)[None, :]], 0
    )
    wv_aug = np.concatenate(
        [np.asarray(inputs["wv"], np.float32), np.asarray(inputs["bv"], np.float32)[None, :]], 0
    )
    shared = {
        "wq": wq_aug, "wk": wk_aug, "wvT": np.ascontiguousarray(wv_aug.T),
        "wch4": np.ascontiguousarray(np.tile(
            np.asarray(inputs["w_ch"], np.float32).reshape(4, C, C).transpose(1, 0, 2).reshape(C, 4 * C),
            (4, 1),
        )),
        "wpos4": np.ascontiguousarray(np.tile(
            np.asarray(inputs["w_pos"], np.float32).reshape(4, C, C).transpose(1, 0, 2).reshape(C, 4 * C),
            (4, 1),
        )),
        "bch": np.ascontiguousarray(np.asarray(inputs["b_ch"], np.float32)[:, None]),
        "bpos": np.ascontiguousarray(np.asarray(inputs["b_pos"], np.float32)[:, None]),
        "beta": np.full((1, 1), beta[0], np.float32),
        "beta128": np.full((128, 1), beta[0], np.float32),
        "gamma": np.full((C, 1), gamma[0], np.float32),
        "ones": np.ones((1, C), np.float32),
        "id32r": np.eye(C, dtype=np.float32),
        "id32f": np.eye(C, dtype=np.float32),
    }
    in_maps = []
    for core in range(NCORES):
        b, s = core // 2, core % 2
        qs = x[b].reshape(N, C)
        # local-half-first column permutation: S_T rhs slices [0, NL) are the
        # core's own rows; softmax sums over all j are order-invariant.
        q = np.concatenate([qs[s * NL:(s + 1) * NL], qs[(1 - s) * NL:(2 - s) * NL]])
        q_aug = np.concatenate([q, np.ones((N, 1), np.float32)], 1)
        qloc_aug = q_aug[:NL]
        qc = np.ascontiguousarray(q_aug.reshape(NJC, 128, C + 1).transpose(1, 0, 2))
        m = dict(shared)
        qT_host = np.ascontiguousarray(q.T)
        m["qT"] = qT_host.astype(ml_dtypes.bfloat16) if S_BF16 else qT_host
        m["qTloc"] = np.ascontiguousarray(qloc_aug.T)
        m["qTlocf"] = np.ascontiguousarray(qloc_aug.T[:C])
        m["qc2d"] = np.ascontiguousarray(qc[:, :, :C]).astype(ml_dtypes.bfloat16)
        m["qcf"] = qc
        in_maps.append(m)
    return in_maps


def _run(inputs, trace=False):
    nc = _get_nc()
    in_maps = _prepare_in_maps(inputs)
    res = bass_utils.run_bass_kernel_spmd(
        nc, in_maps, core_ids=list(range(NCORES)), trace=trace
    )
    out = np.empty((B, H, W, DO, C), np.float32)
    for core in range(NCORES):
        b, s = core // 2, core % 2
        full = res.results[core]["out"].reshape(8, W, D, C)
        out[b, s * 8:(s + 1) * 8] = full[:, :, :DO, :]
    return out, res


def kernel(**inputs):
    out, _ = _run(inputs, trace=False)
    return out
